# revision 7
# baseline (speedup 1.0000x reference)
"""GCN layer (out = A @ x @ W, A sparse COO) on 8 Trainium2 NeuronCores.

Strategy (1D dest partitioning, x replicated), v3:
  - Destinations (output rows) are sharded across the 8 cores; x (fp32) and
    the [64,64] weight (bf16) are replicated to every core's HBM.
  - Host-side preprocessing is pure indexing: edges are bucketed by
    (core, dest-block of 128 rows, source-chunk of 25000 rows) and padded to
    a per-(block,chunk) slot capacity (max over cores -> one SPMD NEFF).
    The host emits an int16 gather-index stream (pad 0 up to the max real
    count over cores, -1 beyond so the Q7 descriptor generator trims whole
    trailing pad slots; num_idxs_reg carries the shared trimmed count so the
    sequencer's descriptor-ring reservation matches what the Q7 generates),
    plus per-edge val (fp32) and dest-local id (bf16) streams.
  - Device per core, per (dest-window, chunk): DMA idx/val/dst streams;
    dma_gather x rows (256B each) into SBUF, one call per (block,chunk)
    segment, round-robining the 4 SWDGE queues so all four Q7 CPU pairs
    generate descriptors concurrently (~3.4x vs one queue); DVE multiplies
    gathered rows by val (fp32 in, bf16 out) and builds the one-hot
    [128 edge-pos, 128 dest-local] in bf16 via is_equal against an iota row;
    PE accumulates aggT[64,128d] += gv[128e,64]^T @ onehot[128e,128d] per
    block in PSUM across the window; at window end the weight is applied
    per block (out_blk = aggT^T @ W, bf16) and results are DMA'd out fp32.
  - Host concatenates the 8 output shards and truncates padding.
"""

import numpy as np


# ---------------------------------------------------------------- config ---
class CFG:
    def __init__(self, n_nodes, d, n_cores, chunk, nchunks, nblk, window):
        self.N = n_nodes
        self.D = d
        self.C = n_cores
        self.CHUNK = chunk          # x rows per gather chunk (< 32768 for int16)
        self.NCH = nchunks
        assert chunk * nchunks >= n_nodes
        self.NBLK = nblk            # dest blocks (of 128 rows) per core
        self.CORE_ROWS = 128 * nblk
        assert self.CORE_ROWS * n_cores >= n_nodes
        self.WINDOW = window        # blocks per window
        self.windows = [
            (w0, min(w0 + window, nblk)) for w0 in range(0, nblk, window)
        ]


FULL = CFG(n_nodes=100000, d=64, n_cores=8, chunk=25000, nchunks=4,
           nblk=98, window=6)

NQUEUES = 4   # SWDGE queues: 4 Q7 cpu pairs generate descriptors in parallel
PAD_NEG = True  # trim trailing pad slots via -1 indices


# ---------------------------------------------------------- preprocessing ---
def preprocess(x, edge_row, edge_col, edge_val, cfg):
    """Bucket/pad edges; build per-core device input arrays.

    Returns (caps, plan, per_core, TOTS):
      caps[b][k]  : slots (128-edge groups) for (block b, chunk k), shared
                    across cores.
      plan        : list over (w,k) of dicts (w0, w1, k, nslots, slot_block,
                    bcaps, bregs).
      per_core    : list of dicts of numpy arrays keyed by dram tensor name.
    """
    import ml_dtypes

    C, NBLK, NCH = cfg.C, cfg.NBLK, cfg.NCH
    r = edge_row.astype(np.int64)
    c = r // cfg.CORE_ROWS
    rr = r % cfg.CORE_ROWS
    b = rr // 128
    d = rr % 128
    k = edge_col.astype(np.int64) // cfg.CHUNK
    lidx = (edge_col.astype(np.int64) % cfg.CHUNK).astype(np.int16)

    key = ((c * NBLK + b) * NCH + k)
    order = np.argsort(key, kind="stable")
    counts = np.bincount(key[order], minlength=C * NBLK * NCH) \
        .reshape(C, NBLK, NCH)

    caps = np.ceil(counts / 128).astype(np.int64).max(axis=0)  # [NBLK, NCH]
    # every block must own >= 1 slot so its PSUM gets initialized
    empty = caps.sum(axis=1) == 0
    caps[empty, 0] = 1
    maxcnt = counts.max(axis=0)  # [NBLK, NCH] max real count over cores

    lidx_s = lidx[order]
    val_s = edge_val[order].astype(np.float32)
    d_s = d[order].astype(np.int64)

    starts = np.zeros(C * NBLK * NCH + 1, dtype=np.int64)
    np.cumsum(counts.reshape(-1), out=starts[1:])

    plan = []
    for (w0, w1) in cfg.windows:
        for kk in range(NCH):
            nslots = int(caps[w0:w1, kk].sum())
            slot_block = np.repeat(np.arange(w0, w1), caps[w0:w1, kk])
            plan.append(dict(
                w0=w0, w1=w1, k=kk, nslots=nslots, slot_block=slot_block,
                bcaps=[int(caps[bb, kk]) for bb in range(w0, w1)],
                # shared trimmed gather count per (block,chunk) call: all
                # cores pad with idx 0 up to this, -1 beyond, so every core
                # trims to the same 128-bucket as the ring reservation
                bregs=[int(maxcnt[bb, kk]) for bb in range(w0, w1)]))

    TOTS = sum(p["nslots"] for p in plan)

    per_core = []
    for cc in range(C):
        idx_mat = np.full((128, TOTS * 8), -1, dtype=np.int16)
        val_mat = np.zeros((128, TOTS), dtype=np.float32)
        dst_mat = np.zeros((128, TOTS), dtype=ml_dtypes.bfloat16)
        off = 0
        it = 0
        for p in plan:
            n = p["nslots"]
            if n == 0:
                it += 1
                continue
            kk = p["k"]
            seg_idx = np.full(n * 128, -1, dtype=np.int16)
            seg_val = np.zeros(n * 128, dtype=np.float32)
            seg_dst = np.zeros(n * 128, dtype=np.int64)
            pos = 0
            for bi, bb in enumerate(range(p["w0"], p["w1"])):
                gi = (cc * NBLK + bb) * NCH + kk
                s0, s1 = starts[gi], starts[gi + 1]
                cnt = s1 - s0
                seg_idx[pos:pos + cnt] = lidx_s[s0:s1]
                # pad with idx 0 up to the shared trimmed count (or the full
                # capacity in the first iterations / no-trim mode, so the
                # gather buffers never expose SBUF garbage to the cast)
                lim = p["bregs"][bi] if (PAD_NEG and it >= 4) \
                    else int(caps[bb, kk]) * 128
                seg_idx[pos + cnt:pos + lim] = 0
                seg_val[pos:pos + cnt] = val_s[s0:s1]
                seg_dst[pos:pos + cnt] = d_s[s0:s1]
                pos += int(caps[bb, kk]) * 128
            assert pos == n * 128
            # gather idx wrap: stream pos j -> (partition j%16, col j//16),
            # replicated into the 8 groups of 16 partitions
            iw = seg_idx.reshape(n * 8, 16).T          # [16, n*8]
            idx_mat[:, off * 8:(off + n) * 8] = np.tile(iw, (8, 1))
            # val/dst wrap: pos j -> (partition j%128, slot j//128)
            val_mat[:, off:off + n] = seg_val.reshape(n, 128).T
            dst_mat[:, off:off + n] = seg_dst.reshape(n, 128).T
            off += n
            it += 1
        per_core.append(dict(idx=idx_mat, vd=val_mat, db=dst_mat))

    return caps, plan, per_core, TOTS


# ---------------------------------------------------------------- kernel ---
def build_bass(cfg, caps, plan, TOTS):
    import concourse.bacc as bacc
    import concourse.bass as bass
    import concourse.mybir as mybir
    import concourse.tile as tile
    from concourse import library_config
    from concourse._compat import get_trn_type

    f32 = mybir.dt.float32
    bf16 = mybir.dt.bfloat16
    i16 = mybir.dt.int16
    D, NCH = cfg.D, cfg.NCH

    NWKMAX = max(p["nslots"] for p in plan)

    nc = bacc.Bacc(get_trn_type() or "TRN2", target_bir_lowering=False,
                   debug=False, num_swdge_queues=NQUEUES)
    x_hbm = nc.dram_tensor("x", [cfg.CHUNK * NCH, D], f32,
                           kind="ExternalInput")
    w_hbm = nc.dram_tensor("w", [D, D], bf16, kind="ExternalInput")
    iota_hbm = nc.dram_tensor("iota", [128, 128], bf16, kind="ExternalInput")
    idx_hbm = nc.dram_tensor("idx", [128, TOTS * 8], i16,
                             kind="ExternalInput")
    vd_hbm = nc.dram_tensor("vd", [128, TOTS], f32, kind="ExternalInput")
    db_hbm = nc.dram_tensor("db", [128, TOTS], bf16, kind="ExternalInput")
    out_hbm = nc.dram_tensor("out", [cfg.CORE_ROWS, D], f32,
                             kind="ExternalOutput")

    # block -> (first (plan idx, slot), last (plan idx, slot)) for start/stop
    first_slot = {}
    last_slot = {}
    for pi, p in enumerate(plan):
        for s, bb in enumerate(p["slot_block"]):
            bb = int(bb)
            if bb not in first_slot:
                first_slot[bb] = (pi, s)
            last_slot[bb] = (pi, s)

    with tile.TileContext(nc) as tc:
        with (
            tc.tile_pool(name="const", bufs=1) as constp,
            tc.tile_pool(name="idxp", bufs=4) as idxp,
            tc.tile_pool(name="vdp", bufs=4) as vdp,
            tc.tile_pool(name="dbp", bufs=4) as dbp,
            tc.tile_pool(name="gp", bufs=6) as gp,
            tc.tile_pool(name="gvp", bufs=4) as gvp,
            tc.tile_pool(name="sp", bufs=4) as sp,
            tc.tile_pool(name="aggsb", bufs=4) as aggsbp,
            tc.tile_pool(name="stg", bufs=2) as stgp,
            tc.tile_pool(name="aggps", bufs=cfg.WINDOW,
                         space=bass.MemorySpace.PSUM) as aggpsp,
            tc.tile_pool(name="out2ps", bufs=2,
                         space=bass.MemorySpace.PSUM) as out2psp,
        ):
            nc.gpsimd.load_library(library_config.mlp)
            w_sb = constp.tile([D, D], bf16, tag="w")
            nc.sync.dma_start(w_sb[:], w_hbm[:])
            iota_sb = constp.tile([128, 128], bf16, tag="iota")
            nc.sync.dma_start(iota_sb[:], iota_hbm[:])

            qcounter = [0]

            for wi, (w0, w1) in enumerate(cfg.windows):
                nb = w1 - w0
                aggps = [aggpsp.tile([64, 128], f32, tag="aggps",
                                     name=f"aggps_w{wi}_{i}")
                         for i in range(nb)]

                for kk in range(NCH):
                    pi = wi * NCH + kk
                    p = plan[pi]
                    n = p["nslots"]
                    if n == 0:
                        continue
                    off = sum(q["nslots"] for q in plan[:pi])

                    idx_t = idxp.tile([128, NWKMAX * 8], i16, tag="idx")
                    nc.sync.dma_start(idx_t[:, :n * 8],
                                      idx_hbm[:, off * 8:(off + n) * 8])
                    vd_t = vdp.tile([128, NWKMAX], f32, tag="vd")
                    nc.sync.dma_start(vd_t[:, :n], vd_hbm[:, off:off + n])
                    db_t = dbp.tile([128, NWKMAX], bf16, tag="db")
                    nc.sync.dma_start(db_t[:, :n], db_hbm[:, off:off + n])

                    g_t = gp.tile([128, NWKMAX, D], f32, tag="g")
                    # one gather call per (block, chunk) segment so the
                    # trailing -1 pad indices trim whole pad slots; round-
                    # robin the 4 SWDGE queues (4 Q7 cpu pairs in parallel)
                    q0 = 0
                    for bi, bc in enumerate(p["bcaps"]):
                        if bc == 0:
                            continue
                        nq = bc * 128
                        reg = p["bregs"][bi] if (PAD_NEG and pi >= 4) else nq
                        nc.gpsimd.dma_gather(
                            g_t[:, q0:q0 + bc, :],
                            x_hbm[kk * cfg.CHUNK:(kk + 1) * cfg.CHUNK, :],
                            idx_t[:, q0 * 8:(q0 + bc) * 8], nq, reg, D,
                            queue_num=qcounter[0] % NQUEUES)
                        qcounter[0] += 1
                        q0 += bc
                    assert q0 == n

                    gv_t = gvp.tile([128, NWKMAX, D], bf16, tag="gv")
                    nc.vector.tensor_tensor(
                        gv_t[:, :n, :], g_t[:, :n, :],
                        vd_t[:, :n].unsqueeze(2).broadcast_to([128, n, D]),
                        mybir.AluOpType.mult)
                    s_t = sp.tile([128, NWKMAX, 128], bf16, tag="s")
                    nc.vector.tensor_tensor(
                        s_t[:, :n, :],
                        db_t[:, :n].unsqueeze(2).broadcast_to([128, n, 128]),
                        iota_sb[:, :].unsqueeze(1).broadcast_to([128, n, 128]),
                        mybir.AluOpType.is_equal)

                    for s in range(n):
                        bb = int(p["slot_block"][s])
                        st = first_slot[bb] == (pi, s)
                        sp_ = last_slot[bb] == (pi, s)
                        nc.tensor.matmul(
                            aggps[bb - w0][:, :],
                            gv_t[:, s, :],
                            s_t[:, s, :],
                            start=st, stop=sp_,
                            skip_group_check=True)

                # ---- flush window: apply W, stage, DMA out
                stg_t = stgp.tile([128, cfg.WINDOW, D], f32, tag="stg")
                out2 = out2psp.tile([128, cfg.WINDOW, D], f32, tag="out2")
                for bi in range(nb):
                    agg_sb = aggsbp.tile([64, 128], bf16, tag="aggsb",
                                         name=f"aggsb_w{wi}_{bi}")
                    nc.vector.tensor_copy(agg_sb[:, :], aggps[bi][:, :])
                    nc.tensor.matmul(out2[:, bi, :],
                                     agg_sb[:, :], w_sb[:],
                                     start=True, stop=True,
                                     skip_group_check=True)
                nc.vector.tensor_copy(stg_t[:, :nb, :], out2[:, :nb, :])
                nc.sync.dma_start(
                    out_hbm[w0 * 128:w1 * 128, :]
                    .rearrange("(b p) f -> p b f", p=128),
                    stg_t[:, :nb, :])

    nc.compile()
    return nc


# ------------------------------------------------------------------- run ---
def run(x, weight, edge_row, edge_col, edge_val, cfg=FULL, trace=False,
        trace_kwargs=None):
    import ml_dtypes
    from concourse.bass_utils import run_bass_kernel_spmd

    caps, plan, per_core, TOTS = preprocess(x, edge_row, edge_col, edge_val,
                                            cfg)
    nc = build_bass(cfg, caps, plan, TOTS)

    xpad = x
    if cfg.CHUNK * cfg.NCH > cfg.N:
        xpad = np.concatenate(
            [x, np.zeros((cfg.CHUNK * cfg.NCH - cfg.N, cfg.D),
                         dtype=np.float32)], axis=0)
    w16 = weight.astype(ml_dtypes.bfloat16)
    iota = np.tile(np.arange(128, dtype=np.float32), (128, 1)) \
        .astype(ml_dtypes.bfloat16)

    in_maps = []
    for cc in range(cfg.C):
        in_maps.append(dict(x=np.ascontiguousarray(xpad),
                            w=np.ascontiguousarray(w16),
                            iota=iota,
                            idx=per_core[cc]["idx"],
                            vd=per_core[cc]["vd"],
                            db=per_core[cc]["db"]))
    kw = {}
    if trace:
        kw = dict(trace=True, trace_kwargs=trace_kwargs or {})
    res = run_bass_kernel_spmd(nc, in_maps, core_ids=list(range(cfg.C)), **kw)
    outs = [r["out"] for r in res.results]
    full = np.concatenate(outs, axis=0)[:cfg.N]
    return full, res


def kernel(x, weight, edge_row, edge_col, edge_val):
    x = np.asarray(x, dtype=np.float32)
    weight = np.asarray(weight, dtype=np.float32)
    edge_row = np.asarray(edge_row, dtype=np.int32)
    edge_col = np.asarray(edge_col, dtype=np.int32)
    edge_val = np.asarray(edge_val, dtype=np.float32)
    out, _ = run(x, weight, edge_row, edge_col, edge_val, FULL)
    return out


# revision 8
# speedup vs baseline: 1.0526x; 1.0526x over previous
"""GCN layer (out = A @ x @ W, A sparse COO) on 8 Trainium2 NeuronCores.

Strategy (1D dest partitioning, x replicated), v3:
  - Destinations (output rows) are sharded across the 8 cores; x (fp32) and
    the [64,64] weight (bf16) are replicated to every core's HBM.
  - Host-side preprocessing is pure indexing: edges are bucketed by
    (core, dest-block of 128 rows, source-chunk of 25000 rows) and padded to
    a per-(block,chunk) slot capacity (max over cores -> one SPMD NEFF).
    The host emits an int16 gather-index stream (pad 0 up to the max real
    count over cores, -1 beyond so the Q7 descriptor generator trims whole
    trailing pad slots; num_idxs_reg carries the shared trimmed count so the
    sequencer's descriptor-ring reservation matches what the Q7 generates),
    plus per-edge val (fp32) and dest-local id (bf16) streams.
  - Device per core, per (dest-window, chunk): DMA idx/val/dst streams;
    dma_gather x rows (256B each) into SBUF, one call per (block,chunk)
    segment, round-robining the 4 SWDGE queues so all four Q7 CPU pairs
    generate descriptors concurrently (~3.4x vs one queue); DVE multiplies
    gathered rows by val (fp32 in, bf16 out) and builds the one-hot
    [128 edge-pos, 128 dest-local] in bf16 via is_equal against an iota row;
    PE accumulates aggT[64,128d] += gv[128e,64]^T @ onehot[128e,128d] per
    block in PSUM across the window; at window end the weight is applied
    per block (out_blk = aggT^T @ W, bf16) and results are DMA'd out fp32.
  - Host concatenates the 8 output shards and truncates padding.
"""

import numpy as np


# ---------------------------------------------------------------- config ---
class CFG:
    def __init__(self, n_nodes, d, n_cores, chunk, nchunks, nblk, window):
        self.N = n_nodes
        self.D = d
        self.C = n_cores
        self.CHUNK = chunk          # x rows per gather chunk (< 32768 for int16)
        self.NCH = nchunks
        assert chunk * nchunks >= n_nodes
        self.NBLK = nblk            # dest blocks (of 128 rows) per core
        self.CORE_ROWS = 128 * nblk
        assert self.CORE_ROWS * n_cores >= n_nodes
        self.WINDOW = window        # blocks per window
        self.windows = [
            (w0, min(w0 + window, nblk)) for w0 in range(0, nblk, window)
        ]


FULL = CFG(n_nodes=100000, d=64, n_cores=8, chunk=25000, nchunks=4,
           nblk=98, window=6)

NQUEUES = 4   # SWDGE queues: 4 Q7 cpu pairs generate descriptors in parallel
PAD_NEG = False  # merged calls: all pads gather row 0 (no -1 trim)


# ---------------------------------------------------------- preprocessing ---
def preprocess(x, edge_row, edge_col, edge_val, cfg):
    """Bucket/pad edges; build per-core device input arrays.

    Returns (caps, plan, per_core, TOTS):
      caps[b][k]  : slots (128-edge groups) for (block b, chunk k), shared
                    across cores.
      plan        : list over (w,k) of dicts (w0, w1, k, nslots, slot_block,
                    bcaps, bregs).
      per_core    : list of dicts of numpy arrays keyed by dram tensor name.
    """
    import ml_dtypes

    C, NBLK, NCH = cfg.C, cfg.NBLK, cfg.NCH
    r = edge_row.astype(np.int64)
    c = r // cfg.CORE_ROWS
    rr = r % cfg.CORE_ROWS
    b = rr // 128
    d = rr % 128
    k = edge_col.astype(np.int64) // cfg.CHUNK
    lidx = (edge_col.astype(np.int64) % cfg.CHUNK).astype(np.int16)

    key = ((c * NBLK + b) * NCH + k)
    order = np.argsort(key, kind="stable")
    counts = np.bincount(key[order], minlength=C * NBLK * NCH) \
        .reshape(C, NBLK, NCH)

    caps = np.ceil(counts / 128).astype(np.int64).max(axis=0)  # [NBLK, NCH]
    # every block must own >= 1 slot so its PSUM gets initialized
    empty = caps.sum(axis=1) == 0
    caps[empty, 0] = 1
    maxcnt = counts.max(axis=0)  # [NBLK, NCH] max real count over cores

    lidx_s = lidx[order]
    val_s = edge_val[order].astype(np.float32)
    d_s = d[order].astype(np.int64)

    starts = np.zeros(C * NBLK * NCH + 1, dtype=np.int64)
    np.cumsum(counts.reshape(-1), out=starts[1:])

    plan = []
    for (w0, w1) in cfg.windows:
        for kk in range(NCH):
            nslots = int(caps[w0:w1, kk].sum())
            slot_block = np.repeat(np.arange(w0, w1), caps[w0:w1, kk])
            plan.append(dict(
                w0=w0, w1=w1, k=kk, nslots=nslots, slot_block=slot_block,
                bcaps=[int(caps[bb, kk]) for bb in range(w0, w1)],
                # shared trimmed gather count per (block,chunk) call: all
                # cores pad with idx 0 up to this, -1 beyond, so every core
                # trims to the same 128-bucket as the ring reservation
                bregs=[int(maxcnt[bb, kk]) for bb in range(w0, w1)]))

    TOTS = sum(p["nslots"] for p in plan)

    per_core = []
    for cc in range(C):
        idx_mat = np.full((128, TOTS * 8), -1, dtype=np.int16)
        val_mat = np.zeros((128, TOTS), dtype=np.float32)
        dst_mat = np.zeros((128, TOTS), dtype=ml_dtypes.bfloat16)
        off = 0
        it = 0
        for p in plan:
            n = p["nslots"]
            if n == 0:
                it += 1
                continue
            kk = p["k"]
            seg_idx = np.full(n * 128, -1, dtype=np.int16)
            seg_val = np.zeros(n * 128, dtype=np.float32)
            seg_dst = np.zeros(n * 128, dtype=np.int64)
            pos = 0
            for bi, bb in enumerate(range(p["w0"], p["w1"])):
                gi = (cc * NBLK + bb) * NCH + kk
                s0, s1 = starts[gi], starts[gi + 1]
                cnt = s1 - s0
                seg_idx[pos:pos + cnt] = lidx_s[s0:s1]
                # pad with idx 0 up to the shared trimmed count (or the full
                # capacity in the first iterations / no-trim mode, so the
                # gather buffers never expose SBUF garbage to the cast)
                lim = p["bregs"][bi] if (PAD_NEG and it >= 4) \
                    else int(caps[bb, kk]) * 128
                seg_idx[pos + cnt:pos + lim] = 0
                seg_val[pos:pos + cnt] = val_s[s0:s1]
                seg_dst[pos:pos + cnt] = d_s[s0:s1]
                pos += int(caps[bb, kk]) * 128
            assert pos == n * 128
            # gather idx wrap: stream pos j -> (partition j%16, col j//16),
            # replicated into the 8 groups of 16 partitions
            iw = seg_idx.reshape(n * 8, 16).T          # [16, n*8]
            idx_mat[:, off * 8:(off + n) * 8] = np.tile(iw, (8, 1))
            # val/dst wrap: pos j -> (partition j%128, slot j//128)
            val_mat[:, off:off + n] = seg_val.reshape(n, 128).T
            dst_mat[:, off:off + n] = seg_dst.reshape(n, 128).T
            off += n
            it += 1
        per_core.append(dict(idx=idx_mat, vd=val_mat, db=dst_mat))

    return caps, plan, per_core, TOTS


# ---------------------------------------------------------------- kernel ---
def build_bass(cfg, caps, plan, TOTS):
    import concourse.bacc as bacc
    import concourse.bass as bass
    import concourse.mybir as mybir
    import concourse.tile as tile
    from concourse import library_config
    from concourse._compat import get_trn_type

    f32 = mybir.dt.float32
    bf16 = mybir.dt.bfloat16
    i16 = mybir.dt.int16
    D, NCH = cfg.D, cfg.NCH

    NWKMAX = max(p["nslots"] for p in plan)

    nc = bacc.Bacc(get_trn_type() or "TRN2", target_bir_lowering=False,
                   debug=False, num_swdge_queues=NQUEUES)
    x_hbm = nc.dram_tensor("x", [cfg.CHUNK * NCH, D], f32,
                           kind="ExternalInput")
    w_hbm = nc.dram_tensor("w", [D, D], bf16, kind="ExternalInput")
    iota_hbm = nc.dram_tensor("iota", [128, 128], bf16, kind="ExternalInput")
    idx_hbm = nc.dram_tensor("idx", [128, TOTS * 8], i16,
                             kind="ExternalInput")
    vd_hbm = nc.dram_tensor("vd", [128, TOTS], f32, kind="ExternalInput")
    db_hbm = nc.dram_tensor("db", [128, TOTS], bf16, kind="ExternalInput")
    out_hbm = nc.dram_tensor("out", [cfg.CORE_ROWS, D], f32,
                             kind="ExternalOutput")

    # block -> (first (plan idx, slot), last (plan idx, slot)) for start/stop
    first_slot = {}
    last_slot = {}
    for pi, p in enumerate(plan):
        for s, bb in enumerate(p["slot_block"]):
            bb = int(bb)
            if bb not in first_slot:
                first_slot[bb] = (pi, s)
            last_slot[bb] = (pi, s)

    with tile.TileContext(nc) as tc:
        with (
            tc.tile_pool(name="const", bufs=1) as constp,
            tc.tile_pool(name="idxp", bufs=4) as idxp,
            tc.tile_pool(name="vdp", bufs=4) as vdp,
            tc.tile_pool(name="dbp", bufs=4) as dbp,
            tc.tile_pool(name="gp", bufs=6) as gp,
            tc.tile_pool(name="gvp", bufs=4) as gvp,
            tc.tile_pool(name="sp", bufs=4) as sp,
            tc.tile_pool(name="aggsb", bufs=4) as aggsbp,
            tc.tile_pool(name="stg", bufs=2) as stgp,
            tc.tile_pool(name="aggps", bufs=cfg.WINDOW,
                         space=bass.MemorySpace.PSUM) as aggpsp,
            tc.tile_pool(name="out2ps", bufs=2,
                         space=bass.MemorySpace.PSUM) as out2psp,
        ):
            nc.gpsimd.load_library(library_config.mlp)
            w_sb = constp.tile([D, D], bf16, tag="w")
            nc.sync.dma_start(w_sb[:], w_hbm[:])
            iota_sb = constp.tile([128, 128], bf16, tag="iota")
            nc.sync.dma_start(iota_sb[:], iota_hbm[:])

            qcounter = [0]

            for wi, (w0, w1) in enumerate(cfg.windows):
                nb = w1 - w0
                aggps = [aggpsp.tile([64, 128], f32, tag="aggps",
                                     name=f"aggps_w{wi}_{i}")
                         for i in range(nb)]

                for kk in range(NCH):
                    pi = wi * NCH + kk
                    p = plan[pi]
                    n = p["nslots"]
                    if n == 0:
                        continue
                    off = sum(q["nslots"] for q in plan[:pi])

                    idx_t = idxp.tile([128, NWKMAX * 8], i16, tag="idx")
                    nc.sync.dma_start(idx_t[:, :n * 8],
                                      idx_hbm[:, off * 8:(off + n) * 8])
                    vd_t = vdp.tile([128, NWKMAX], f32, tag="vd")
                    nc.sync.dma_start(vd_t[:, :n], vd_hbm[:, off:off + n])
                    db_t = dbp.tile([128, NWKMAX], bf16, tag="db")
                    nc.sync.dma_start(db_t[:, :n], db_hbm[:, off:off + n])

                    g_t = gp.tile([128, NWKMAX, D], f32, tag="g")
                    # one gather call per (block, chunk) segment so the
                    # trailing -1 pad indices trim whole pad slots; round-
                    # robin the 4 SWDGE queues (4 Q7 cpu pairs in parallel)
                    for q0 in range(0, n, 8):
                        bc = min(8, n - q0)
                        nq = bc * 128
                        nc.gpsimd.dma_gather(
                            g_t[:, q0:q0 + bc, :],
                            x_hbm[kk * cfg.CHUNK:(kk + 1) * cfg.CHUNK, :],
                            idx_t[:, q0 * 8:(q0 + bc) * 8], nq, nq, D,
                            queue_num=qcounter[0] % NQUEUES)
                        qcounter[0] += 1

                    gv_t = gvp.tile([128, NWKMAX, D], bf16, tag="gv")
                    nc.vector.tensor_tensor(
                        gv_t[:, :n, :], g_t[:, :n, :],
                        vd_t[:, :n].unsqueeze(2).broadcast_to([128, n, D]),
                        mybir.AluOpType.mult)
                    s_t = sp.tile([128, NWKMAX, 128], bf16, tag="s")
                    nc.vector.tensor_tensor(
                        s_t[:, :n, :],
                        db_t[:, :n].unsqueeze(2).broadcast_to([128, n, 128]),
                        iota_sb[:, :].unsqueeze(1).broadcast_to([128, n, 128]),
                        mybir.AluOpType.is_equal)

                    for s in range(n):
                        bb = int(p["slot_block"][s])
                        st = first_slot[bb] == (pi, s)
                        sp_ = last_slot[bb] == (pi, s)
                        nc.tensor.matmul(
                            aggps[bb - w0][:, :],
                            gv_t[:, s, :],
                            s_t[:, s, :],
                            start=st, stop=sp_,
                            skip_group_check=True)

                # ---- flush window: apply W, stage, DMA out
                stg_t = stgp.tile([128, cfg.WINDOW, D], f32, tag="stg")
                out2 = out2psp.tile([128, cfg.WINDOW, D], f32, tag="out2")
                for bi in range(nb):
                    agg_sb = aggsbp.tile([64, 128], bf16, tag="aggsb",
                                         name=f"aggsb_w{wi}_{bi}")
                    nc.vector.tensor_copy(agg_sb[:, :], aggps[bi][:, :])
                    nc.tensor.matmul(out2[:, bi, :],
                                     agg_sb[:, :], w_sb[:],
                                     start=True, stop=True,
                                     skip_group_check=True)
                nc.vector.tensor_copy(stg_t[:, :nb, :], out2[:, :nb, :])
                nc.sync.dma_start(
                    out_hbm[w0 * 128:w1 * 128, :]
                    .rearrange("(b p) f -> p b f", p=128),
                    stg_t[:, :nb, :])

    nc.compile()
    return nc


# ------------------------------------------------------------------- run ---
def run(x, weight, edge_row, edge_col, edge_val, cfg=FULL, trace=False,
        trace_kwargs=None):
    import ml_dtypes
    from concourse.bass_utils import run_bass_kernel_spmd

    caps, plan, per_core, TOTS = preprocess(x, edge_row, edge_col, edge_val,
                                            cfg)
    nc = build_bass(cfg, caps, plan, TOTS)

    xpad = x
    if cfg.CHUNK * cfg.NCH > cfg.N:
        xpad = np.concatenate(
            [x, np.zeros((cfg.CHUNK * cfg.NCH - cfg.N, cfg.D),
                         dtype=np.float32)], axis=0)
    w16 = weight.astype(ml_dtypes.bfloat16)
    iota = np.tile(np.arange(128, dtype=np.float32), (128, 1)) \
        .astype(ml_dtypes.bfloat16)

    in_maps = []
    for cc in range(cfg.C):
        in_maps.append(dict(x=np.ascontiguousarray(xpad),
                            w=np.ascontiguousarray(w16),
                            iota=iota,
                            idx=per_core[cc]["idx"],
                            vd=per_core[cc]["vd"],
                            db=per_core[cc]["db"]))
    kw = {}
    if trace:
        kw = dict(trace=True, trace_kwargs=trace_kwargs or {})
    res = run_bass_kernel_spmd(nc, in_maps, core_ids=list(range(cfg.C)), **kw)
    outs = [r["out"] for r in res.results]
    full = np.concatenate(outs, axis=0)[:cfg.N]
    return full, res


def kernel(x, weight, edge_row, edge_col, edge_val):
    x = np.asarray(x, dtype=np.float32)
    weight = np.asarray(weight, dtype=np.float32)
    edge_row = np.asarray(edge_row, dtype=np.int32)
    edge_col = np.asarray(edge_col, dtype=np.int32)
    edge_val = np.asarray(edge_val, dtype=np.float32)
    out, _ = run(x, weight, edge_row, edge_col, edge_val, FULL)
    return out


# revision 9
# speedup vs baseline: 1.1268x; 1.0705x over previous
"""GCN layer (out = A @ x @ W, A sparse COO) on 8 Trainium2 NeuronCores.

Strategy (1D dest partitioning, x replicated), v3:
  - Destinations (output rows) are sharded across the 8 cores; x (fp32) and
    the [64,64] weight (bf16) are replicated to every core's HBM.
  - Host-side preprocessing is pure indexing: edges are bucketed by
    (core, dest-block of 128 rows, source-chunk of 25000 rows) and padded to
    a per-(block,chunk) slot capacity (max over cores -> one SPMD NEFF).
    The host emits an int16 gather-index stream (pad 0 -> gathers row 0,
    killed by the zero one-hot columns), plus per-edge val (fp32) and
    dest-local id (bf16) streams.
  - Device per core, per (dest-window, chunk): DMA idx/val/dst streams;
    dma_gather x rows (256B each) into SBUF in 1024-index calls (the
    per-call fixed cost on the Q7 dominates, so fewer/bigger calls win;
    >1024 overflows the descriptor rings), round-robining the 4 SWDGE
    queues so all four Q7 CPU pairs generate descriptors concurrently
    (~3.4x vs one queue); DVE multiplies
    gathered rows by val (fp32 in, bf16 out) and builds the one-hot
    [128 edge-pos, 128 dest-local] in bf16 via is_equal against an iota row;
    PE accumulates aggT[64,128d] += gv[128e,64]^T @ onehot[128e,128d] per
    block in PSUM across the window; at window end the weight is applied
    per block (out_blk = aggT^T @ W, bf16) and results are DMA'd out fp32.
  - Host concatenates the 8 output shards and truncates padding.
"""

import numpy as np


# ---------------------------------------------------------------- config ---
class CFG:
    def __init__(self, n_nodes, d, n_cores, chunk, nchunks, nblk, window):
        self.N = n_nodes
        self.D = d
        self.C = n_cores
        self.CHUNK = chunk          # x rows per gather chunk (< 32768 for int16)
        self.NCH = nchunks
        assert chunk * nchunks >= n_nodes
        self.NBLK = nblk            # dest blocks (of 128 rows) per core
        self.CORE_ROWS = 128 * nblk
        assert self.CORE_ROWS * n_cores >= n_nodes
        self.WINDOW = window        # blocks per window
        self.windows = [
            (w0, min(w0 + window, nblk)) for w0 in range(0, nblk, window)
        ]


FULL = CFG(n_nodes=100000, d=64, n_cores=8, chunk=25000, nchunks=4,
           nblk=98, window=6)

NQUEUES = 4   # SWDGE queues: 4 Q7 cpu pairs generate descriptors in parallel
PAD_NEG = False  # merged calls: all pads gather row 0 (no -1 trim)


# ---------------------------------------------------------- preprocessing ---
def preprocess(x, edge_row, edge_col, edge_val, cfg):
    """Bucket/pad edges; build per-core device input arrays.

    Returns (caps, plan, per_core, TOTS):
      caps[b][k]  : slots (128-edge groups) for (block b, chunk k), shared
                    across cores.
      plan        : list over (w,k) of dicts (w0, w1, k, nslots, slot_block,
                    bcaps, bregs).
      per_core    : list of dicts of numpy arrays keyed by dram tensor name.
    """
    import ml_dtypes

    C, NBLK, NCH = cfg.C, cfg.NBLK, cfg.NCH
    r = edge_row.astype(np.int64)
    c = r // cfg.CORE_ROWS
    rr = r % cfg.CORE_ROWS
    b = rr // 128
    d = rr % 128
    k = edge_col.astype(np.int64) // cfg.CHUNK
    lidx = (edge_col.astype(np.int64) % cfg.CHUNK).astype(np.int16)

    key = ((c * NBLK + b) * NCH + k)
    order = np.argsort(key, kind="stable")
    counts = np.bincount(key[order], minlength=C * NBLK * NCH) \
        .reshape(C, NBLK, NCH)

    caps = np.ceil(counts / 128).astype(np.int64).max(axis=0)  # [NBLK, NCH]
    # every block must own >= 1 slot so its PSUM gets initialized
    empty = caps.sum(axis=1) == 0
    caps[empty, 0] = 1
    maxcnt = counts.max(axis=0)  # [NBLK, NCH] max real count over cores

    lidx_s = lidx[order]
    val_s = edge_val[order].astype(np.float32)
    d_s = d[order].astype(np.int64)

    starts = np.zeros(C * NBLK * NCH + 1, dtype=np.int64)
    np.cumsum(counts.reshape(-1), out=starts[1:])

    plan = []
    for (w0, w1) in cfg.windows:
        for kk in range(NCH):
            nslots = int(caps[w0:w1, kk].sum())
            slot_block = np.repeat(np.arange(w0, w1), caps[w0:w1, kk])
            plan.append(dict(
                w0=w0, w1=w1, k=kk, nslots=nslots, slot_block=slot_block,
                bcaps=[int(caps[bb, kk]) for bb in range(w0, w1)],
                # shared trimmed gather count per (block,chunk) call: all
                # cores pad with idx 0 up to this, -1 beyond, so every core
                # trims to the same 128-bucket as the ring reservation
                bregs=[int(maxcnt[bb, kk]) for bb in range(w0, w1)]))

    TOTS = sum(p["nslots"] for p in plan)

    per_core = []
    for cc in range(C):
        idx_mat = np.full((128, TOTS * 8), -1, dtype=np.int16)
        val_mat = np.zeros((128, TOTS), dtype=np.float32)
        dst_mat = np.zeros((128, TOTS), dtype=ml_dtypes.bfloat16)
        off = 0
        it = 0
        for p in plan:
            n = p["nslots"]
            if n == 0:
                it += 1
                continue
            kk = p["k"]
            seg_idx = np.full(n * 128, -1, dtype=np.int16)
            seg_val = np.zeros(n * 128, dtype=np.float32)
            seg_dst = np.zeros(n * 128, dtype=np.int64)
            pos = 0
            for bi, bb in enumerate(range(p["w0"], p["w1"])):
                gi = (cc * NBLK + bb) * NCH + kk
                s0, s1 = starts[gi], starts[gi + 1]
                cnt = s1 - s0
                seg_idx[pos:pos + cnt] = lidx_s[s0:s1]
                # pad with idx 0 up to the shared trimmed count (or the full
                # capacity in the first iterations / no-trim mode, so the
                # gather buffers never expose SBUF garbage to the cast)
                lim = p["bregs"][bi] if (PAD_NEG and it >= 4) \
                    else int(caps[bb, kk]) * 128
                seg_idx[pos + cnt:pos + lim] = 0
                seg_val[pos:pos + cnt] = val_s[s0:s1]
                seg_dst[pos:pos + cnt] = d_s[s0:s1]
                pos += int(caps[bb, kk]) * 128
            assert pos == n * 128
            # gather idx wrap: stream pos j -> (partition j%16, col j//16),
            # replicated into the 8 groups of 16 partitions
            iw = seg_idx.reshape(n * 8, 16).T          # [16, n*8]
            idx_mat[:, off * 8:(off + n) * 8] = np.tile(iw, (8, 1))
            # val/dst wrap: pos j -> (partition j%128, slot j//128)
            val_mat[:, off:off + n] = seg_val.reshape(n, 128).T
            dst_mat[:, off:off + n] = seg_dst.reshape(n, 128).T
            off += n
            it += 1
        per_core.append(dict(idx=idx_mat, vd=val_mat, db=dst_mat))

    return caps, plan, per_core, TOTS


# ---------------------------------------------------------------- kernel ---
def build_bass(cfg, caps, plan, TOTS):
    import concourse.bacc as bacc
    import concourse.bass as bass
    import concourse.mybir as mybir
    import concourse.tile as tile
    from concourse import library_config
    from concourse._compat import get_trn_type

    f32 = mybir.dt.float32
    bf16 = mybir.dt.bfloat16
    i16 = mybir.dt.int16
    D, NCH = cfg.D, cfg.NCH

    NWKMAX = max(p["nslots"] for p in plan)

    nc = bacc.Bacc(get_trn_type() or "TRN2", target_bir_lowering=False,
                   debug=False, num_swdge_queues=NQUEUES)
    x_hbm = nc.dram_tensor("x", [cfg.CHUNK * NCH, D], f32,
                           kind="ExternalInput")
    w_hbm = nc.dram_tensor("w", [D, D], bf16, kind="ExternalInput")
    iota_hbm = nc.dram_tensor("iota", [128, 128], bf16, kind="ExternalInput")
    idx_hbm = nc.dram_tensor("idx", [128, TOTS * 8], i16,
                             kind="ExternalInput")
    vd_hbm = nc.dram_tensor("vd", [128, TOTS], f32, kind="ExternalInput")
    db_hbm = nc.dram_tensor("db", [128, TOTS], bf16, kind="ExternalInput")
    out_hbm = nc.dram_tensor("out", [cfg.CORE_ROWS, D], f32,
                             kind="ExternalOutput")

    # block -> (first (plan idx, slot), last (plan idx, slot)) for start/stop
    first_slot = {}
    last_slot = {}
    for pi, p in enumerate(plan):
        for s, bb in enumerate(p["slot_block"]):
            bb = int(bb)
            if bb not in first_slot:
                first_slot[bb] = (pi, s)
            last_slot[bb] = (pi, s)

    with tile.TileContext(nc) as tc:
        with (
            tc.tile_pool(name="const", bufs=1) as constp,
            tc.tile_pool(name="idxp", bufs=4) as idxp,
            tc.tile_pool(name="vdp", bufs=4) as vdp,
            tc.tile_pool(name="dbp", bufs=4) as dbp,
            tc.tile_pool(name="gp", bufs=6) as gp,
            tc.tile_pool(name="gvp", bufs=4) as gvp,
            tc.tile_pool(name="sp", bufs=4) as sp,
            tc.tile_pool(name="aggsb", bufs=4) as aggsbp,
            tc.tile_pool(name="stg", bufs=2) as stgp,
            tc.tile_pool(name="aggps", bufs=cfg.WINDOW,
                         space=bass.MemorySpace.PSUM) as aggpsp,
            tc.tile_pool(name="out2ps", bufs=2,
                         space=bass.MemorySpace.PSUM) as out2psp,
        ):
            nc.gpsimd.load_library(library_config.mlp)
            w_sb = constp.tile([D, D], bf16, tag="w")
            nc.sync.dma_start(w_sb[:], w_hbm[:])
            iota_sb = constp.tile([128, 128], bf16, tag="iota")
            nc.sync.dma_start(iota_sb[:], iota_hbm[:])

            qcounter = [0]

            for wi, (w0, w1) in enumerate(cfg.windows):
                nb = w1 - w0
                aggps = [aggpsp.tile([64, 128], f32, tag="aggps",
                                     name=f"aggps_w{wi}_{i}")
                         for i in range(nb)]

                for kk in range(NCH):
                    pi = wi * NCH + kk
                    p = plan[pi]
                    n = p["nslots"]
                    if n == 0:
                        continue
                    off = sum(q["nslots"] for q in plan[:pi])

                    idx_t = idxp.tile([128, NWKMAX * 8], i16, tag="idx")
                    nc.sync.dma_start(idx_t[:, :n * 8],
                                      idx_hbm[:, off * 8:(off + n) * 8])
                    vd_t = vdp.tile([128, NWKMAX], f32, tag="vd")
                    nc.sync.dma_start(vd_t[:, :n], vd_hbm[:, off:off + n])
                    db_t = dbp.tile([128, NWKMAX], bf16, tag="db")
                    nc.sync.dma_start(db_t[:, :n], db_hbm[:, off:off + n])

                    g_t = gp.tile([128, NWKMAX, D], f32, tag="g")
                    # one gather call per (block, chunk) segment so the
                    # trailing -1 pad indices trim whole pad slots; round-
                    # robin the 4 SWDGE queues (4 Q7 cpu pairs in parallel)
                    for q0 in range(0, n, 8):
                        bc = min(8, n - q0)
                        nq = bc * 128
                        nc.gpsimd.dma_gather(
                            g_t[:, q0:q0 + bc, :],
                            x_hbm[kk * cfg.CHUNK:(kk + 1) * cfg.CHUNK, :],
                            idx_t[:, q0 * 8:(q0 + bc) * 8], nq, nq, D,
                            queue_num=qcounter[0] % NQUEUES)
                        qcounter[0] += 1

                    gv_t = gvp.tile([128, NWKMAX, D], bf16, tag="gv")
                    nc.vector.tensor_tensor(
                        gv_t[:, :n, :], g_t[:, :n, :],
                        vd_t[:, :n].unsqueeze(2).broadcast_to([128, n, D]),
                        mybir.AluOpType.mult)
                    s_t = sp.tile([128, NWKMAX, 128], bf16, tag="s")
                    nc.vector.tensor_tensor(
                        s_t[:, :n, :],
                        db_t[:, :n].unsqueeze(2).broadcast_to([128, n, 128]),
                        iota_sb[:, :].unsqueeze(1).broadcast_to([128, n, 128]),
                        mybir.AluOpType.is_equal)

                    for s in range(n):
                        bb = int(p["slot_block"][s])
                        st = first_slot[bb] == (pi, s)
                        sp_ = last_slot[bb] == (pi, s)
                        nc.tensor.matmul(
                            aggps[bb - w0][:, :],
                            gv_t[:, s, :],
                            s_t[:, s, :],
                            start=st, stop=sp_,
                            skip_group_check=True)

                # ---- flush window: apply W, stage, DMA out
                stg_t = stgp.tile([128, cfg.WINDOW, D], f32, tag="stg")
                out2 = out2psp.tile([128, cfg.WINDOW, D], f32, tag="out2")
                for bi in range(nb):
                    agg_sb = aggsbp.tile([64, 128], bf16, tag="aggsb",
                                         name=f"aggsb_w{wi}_{bi}")
                    nc.vector.tensor_copy(agg_sb[:, :], aggps[bi][:, :])
                    nc.tensor.matmul(out2[:, bi, :],
                                     agg_sb[:, :], w_sb[:],
                                     start=True, stop=True,
                                     skip_group_check=True)
                nc.vector.tensor_copy(stg_t[:, :nb, :], out2[:, :nb, :])
                nc.sync.dma_start(
                    out_hbm[w0 * 128:w1 * 128, :]
                    .rearrange("(b p) f -> p b f", p=128),
                    stg_t[:, :nb, :])

    nc.compile()
    return nc


# ------------------------------------------------------------------- run ---
def run(x, weight, edge_row, edge_col, edge_val, cfg=FULL, trace=False,
        trace_kwargs=None):
    import ml_dtypes
    from concourse.bass_utils import run_bass_kernel_spmd

    caps, plan, per_core, TOTS = preprocess(x, edge_row, edge_col, edge_val,
                                            cfg)
    nc = build_bass(cfg, caps, plan, TOTS)

    xpad = x
    if cfg.CHUNK * cfg.NCH > cfg.N:
        xpad = np.concatenate(
            [x, np.zeros((cfg.CHUNK * cfg.NCH - cfg.N, cfg.D),
                         dtype=np.float32)], axis=0)
    w16 = weight.astype(ml_dtypes.bfloat16)
    iota = np.tile(np.arange(128, dtype=np.float32), (128, 1)) \
        .astype(ml_dtypes.bfloat16)

    in_maps = []
    for cc in range(cfg.C):
        in_maps.append(dict(x=np.ascontiguousarray(xpad),
                            w=np.ascontiguousarray(w16),
                            iota=iota,
                            idx=per_core[cc]["idx"],
                            vd=per_core[cc]["vd"],
                            db=per_core[cc]["db"]))
    kw = {}
    if trace:
        kw = dict(trace=True, trace_kwargs=trace_kwargs or {})
    res = run_bass_kernel_spmd(nc, in_maps, core_ids=list(range(cfg.C)), **kw)
    outs = [r["out"] for r in res.results]
    full = np.concatenate(outs, axis=0)[:cfg.N]
    return full, res


def kernel(x, weight, edge_row, edge_col, edge_val):
    x = np.asarray(x, dtype=np.float32)
    weight = np.asarray(weight, dtype=np.float32)
    edge_row = np.asarray(edge_row, dtype=np.int32)
    edge_col = np.asarray(edge_col, dtype=np.int32)
    edge_val = np.asarray(edge_val, dtype=np.float32)
    out, _ = run(x, weight, edge_row, edge_col, edge_val, FULL)
    return out
